# revision 2
# baseline (speedup 1.0000x reference)
"""Trainium2 Bass kernel for BasicQuantConv2d (sync-BN + HWGQ + gauss-quant + 3x3 conv).

Strategy (8 NeuronCores, data-parallel over batch):
  - Each core takes 4 of the 32 images: x shard [4, 128, 56, 56].
  - BN batch stats: per-core bn_stats/bn_aggr -> (mean, E[x^2]) payload,
    AllGather across the 8 cores + local 8-way sum (sync-BN; AllGather is
    ~2x cheaper than AllReduce for a 1KB payload), then the exact /8 is a
    power-of-two scale. Per-channel scale/bias follow.
  - BN + HWGQ folds to ia = RNE_round(clip(x*s_c + b_c, 0, 3)) in {0..3};
    RNE rounding via the fp32 magic constant 1.5*2^23 (matches jnp.round).
  - gauss_quantize(w) == iw * (step/2) with iw in {-3,-1,1,3}; std(w) is
    computed on-device (reduction + ones-matmul broadcast + Newton-refined
    rsqrt), weights transposed per-tap on the PE for the conv lhsT.
  - The 3x3 conv runs in fp8e4m3 (ia in {0..3}, iw in {-3,-1,1,3} are exact
    in fp8; PSUM accumulates fp32 => conv is EXACT integer arithmetic).
    Per output row-chunk: 5 passes -- 3 DoubleRow matmuls (vertical tap
    pairs kh=0&1 per kw, pair-step 64B via the padded row width), 1
    DoubleRow pairing (2,0)+(2,1) horizontally (pair-step 1B), and 1 plain
    fp8 matmul for (2,2) -- accumulated into 7 PSUM banks per image.
  - Output = PSUM * (0.538*step/2) via ScalarE, 896-col DMA out.

Pipelining: `_build(n_iters=K)` emits a prelude (weight path, pad memsets,
gamma/beta) once, then K software-pipelined iterations: each loop round
emits the NEXT iteration's loads+stats (front) before this iteration's
reduce/chain/phase C (back), so the in-order DVE queue processes next-
iteration bn_stats during this iteration's collective. x tiles are double-
buffered (bufs=2), all BN affines run before the per-image clip/round/conv
so x buffers release early, DMA queues are split (x loads on sync, outputs
on scalar, payload/gather on gpsimd), and per-image "bridge" PE fillers keep
the tensor engine's p-state ramp hot across inter-image dependency gaps.
Output DMAs move fp16 (host casts back to fp32); that halves the output
stream at <=2^-11 added relative error. test.py measures per-iteration
device time through the ~80ms axon RPC floor.
"""

import numpy as np

import concourse.bacc as bacc
import concourse.bass as bass
import concourse.tile as tile
from concourse import mybir
from concourse.masks import make_identity

N_CORES = 8
IMG = 4            # images per core
C = 128            # channels (= partitions)
HW = 56
S = HW * HW        # 3136 pixels per image
G = 448            # stats/affine granule (8 image rows)
NT = HW // 8       # 7 granules per image
PR = 58            # padded rows
PCW = 64           # padded row width (interior at cols 2..57; pair-step 64B for DoubleRow)
R = 8              # output rows per matmul tile
NFREE = R * HW     # 448 matmul free dim

HWGQ_STEP = 0.538
GAUSS = 0.996
BN_EPS = 1e-3
MAGIC = float(np.float32(1.5 * 2**23))
NW = 128 * 128 * 9          # weight element count

N_FILL = 14        # PE filler matmuls per iteration (p-state bridge)
N_BRIDGE = 14      # per-image PE fillers bridging inter-image a_t gaps
X_NEWTON = 2       # rsqrt Newton iterations for the BN scale (critical path)

_CACHE = {}


def _emit_prelude(nc, tc, pools, params):
    """Iteration-invariant work: weight quantization, pad memsets, gamma/beta."""
    fp32 = mybir.dt.float32
    fp8 = mybir.dt.float8e4
    xp, apadp, wp, tmpp, outp, smallp, psump, psmallp, dramp = pools
    x_d, gamma_d, beta_d, w_d, y_d = params
    AF = mybir.ActivationFunctionType
    OP = mybir.AluOpType

    w_sb = wp.tile([C, 128 * 9], fp32)
    nc.sync.dma_start(out=w_sb[:], in_=w_d.ap())

    ident = smallp.tile([C, 128], fp32, tag="ident")
    make_identity(nc, ident[:])

    # one 2KB PSUM bank sliced 4 ways: transpose ping/pong, pg, fillers —
    # disjoint slices so the tile dep tracker doesn't serialize the pipeline
    psm = psmallp.tile([C, 512], fp32, tag="psm", name="psm")

    # transpose each tap: wT[ci, slot, co]; slots pair (kh=0,kw) with (kh=1,kw)
    # adjacently for DoubleRow; kh=2 taps in slots 6..8 ((2,0),(2,1) pair too).
    # slot order: (0,0),(1,0),(0,1),(1,1),(0,2),(1,2),(2,0),(2,1),(2,2)
    SLOT = {(0, 0): 0, (1, 0): 1, (0, 1): 2, (1, 1): 3,
            (0, 2): 4, (1, 2): 5, (2, 0): 6, (2, 1): 7, (2, 2): 8}
    wT = wp.tile([C, 9, 128], fp32)
    w3 = w_sb[:].rearrange("p (ci t) -> p ci t", t=9)
    for t in range(9):
        kh, kw = divmod(t, 3)
        pt = psm[:, (t % 2) * 128:(t % 2) * 128 + 128]
        nc.tensor.transpose(pt, w3[:, :, t], ident[:])
        nc.scalar.copy(out=wT[:, SLOT[(kh, kw)], :], in_=pt)

    # global sum / sumsq of w: ScalarE accum_out row-sums + ones-matmul bcast
    # (scratch shares the uw buffer -- both are prelude-only, used serially)
    uw = wp.tile([C, 9, 128], fp32)
    w2_sb = uw[:].rearrange("p a b -> p (a b)")
    rsums = smallp.tile([C, 2], fp32, tag="rsums")
    nc.scalar.activation(out=w2_sb, in_=w_sb[:], func=AF.Identity,
                         accum_out=rsums[:, 0:1])
    nc.scalar.activation(out=w2_sb, in_=w_sb[:], func=AF.Square,
                         accum_out=rsums[:, 1:2])
    ones = smallp.tile([C, 128], fp32, tag="ones")
    nc.vector.memset(ones[:], 1.0)
    pg = psm[:, 256:384]
    nc.tensor.matmul(pg[:, 0:2], lhsT=ones[:], rhs=rsums[:], start=True, stop=True)
    gs = smallp.tile([C, 2], fp32, tag="gs")
    nc.vector.tensor_copy(gs[:], pg[:, 0:2])

    # wvar = E[w^2] - E[w]^2 ; rw = rsqrt(wvar) Newton-refined
    wmean = smallp.tile([C, 1], fp32, tag="wmean")
    wvar = smallp.tile([C, 1], fp32, tag="wvar")
    nc.vector.tensor_scalar_mul(wmean[:], gs[:, 0:1], 1.0 / NW)
    nc.vector.tensor_scalar_mul(wvar[:], gs[:, 1:2], 1.0 / NW)
    wm2 = smallp.tile([C, 1], fp32, tag="wm2")
    nc.vector.tensor_mul(wm2[:], wmean[:], wmean[:])
    nc.vector.tensor_sub(wvar[:], wvar[:], wm2[:])

    rw = smallp.tile([C, 1], fp32, tag="rw")
    nc.scalar.activation(out=rw[:], in_=wvar[:], func=AF.Sqrt)
    nc.vector.reciprocal(out=rw[:], in_=rw[:])
    tN = smallp.tile([C, 1], fp32, tag="tN")
    for _ in range(2):
        nc.vector.tensor_mul(tN[:], rw[:], rw[:])
        nc.vector.tensor_mul(tN[:], wvar[:], tN[:])
        nc.vector.tensor_scalar(tN[:], tN[:], -0.5, 1.5, OP.mult, OP.add)
        nc.vector.tensor_mul(rw[:], rw[:], tN[:])

    inv_step = smallp.tile([C, 1], fp32, tag="inv_step")
    nc.vector.tensor_scalar_mul(inv_step[:], rw[:], 1.0 / GAUSS)
    # alpha = 0.538 * step/2 = (0.538*0.996/2) * wvar * rw
    alpha = smallp.tile([C, 1], fp32, tag="alpha")
    nc.vector.tensor_mul(alpha[:], wvar[:], rw[:])
    nc.vector.tensor_scalar_mul(alpha[:], alpha[:], HWGQ_STEP * GAUSS / 2.0)

    # quantize transposed weights -> iw in {-3,-1,1,3} (fp8)
    nc.gpsimd.tensor_scalar(uw[:], wT[:], inv_step[:], 0.5, OP.mult, OP.add)
    nc.gpsimd.tensor_scalar(uw[:], uw[:], MAGIC, MAGIC, OP.add, OP.subtract)
    nc.gpsimd.tensor_scalar(uw[:], uw[:], 2.0, -1.0, OP.mult, OP.add)
    wq = wp.tile([C, 9, 128], fp8)
    nc.gpsimd.tensor_scalar(wq[:], uw[:], 3.0, -3.0, OP.min, OP.max)

    # gamma/beta (iteration-invariant)
    gb = smallp.tile([C, 2], fp32, tag="gb")
    gamma_ap = gamma_d.ap().rearrange("(p one) -> p one", one=1)
    beta_ap = beta_d.ap().rearrange("(p one) -> p one", one=1)
    nc.sync.dma_start(out=gb[:, 0:1], in_=gamma_ap)
    nc.sync.dma_start(out=gb[:, 1:2], in_=beta_ap)

    # padded fp8 activation tiles: interior is rewritten every iteration,
    # borders stay zero forever -> memset once here
    a_t = [apadp.tile([C, PR, PCW], fp8, tag=f"a{i}", name=f"a_t{i}")
           for i in range(IMG)]
    for i in range(IMG):
        nc.gpsimd.memset(a_t[i][:], 0.0)

    return dict(ones=ones, wq=wq, alpha=alpha, gb=gb, a_t=a_t, psm=psm)


def _emit_front(nc, tc, pools, params, ablate=()):
    """Loads + BN stats + payload for one iteration (emitted one iteration
    ahead so next-iteration stats fill the DVE queue during this iteration's
    collective)."""
    fp32 = mybir.dt.float32
    xp, apadp, wp, tmpp, outp, smallp, psump, psmallp, dramp = pools
    x_d, gamma_d, beta_d, w_d, y_d = params
    OP = mybir.AluOpType

    # ---------------- load x (896-col tiles, 448-col granule views) --------
    # x loads ride the sync (SP) queue -- a dedicated issuer that is never
    # blocked by compute and holds nothing but loads, so iteration i+1's
    # loads dispatch as soon as their (double-buffered) tiles free up
    xH = [[xp.tile([C, 896 if h < 3 else G], fp32, tag=f"x{i}_{h}",
                   name=f"x{i}_{h}") for h in range(4)] for i in range(IMG)]
    for i in range(IMG):
        for h in range(4):
            lo, hi = h * 896, min((h + 1) * 896, S)
            nc.sync.dma_start(out=xH[i][h][:], in_=x_d.ap()[i][:, lo:hi])

    def xgran(i, g):
        t = xH[i][g // 2]
        if g % 2 == 0:
            return t[:, 0:G]
        return t[:, G:2 * G]

    stats = smallp.tile([C, IMG * NT, 6], fp32)
    for i in range(IMG):
        for g in range(NT):
            nc.vector.bn_stats(out=stats[:, i * NT + g, :], in_=xgran(i, g))
    # payload: (mean, E[x^2]) raw; E[x^2] = var + mean^2 (in-place on pay)
    pay = smallp.tile([C, 2], fp32)
    nc.vector.bn_aggr(out=pay[:], in_=stats[:])
    m2 = smallp.tile([C, 1], fp32)
    nc.vector.tensor_mul(m2[:], pay[:, 0:1], pay[:, 0:1])
    nc.vector.tensor_add(pay[:, 1:2], pay[:, 1:2], m2[:])

    return dict(xH=xH, xgran=xgran, pay=pay)


def _emit_dispatch(nc, pools, st, ablate=()):
    """Collective dispatch (gpsimd queue): payload out, AllGather, gather-in."""
    fp32 = mybir.dt.float32
    xp, apadp, wp, tmpp, outp, smallp, psump, psmallp, dramp = pools
    OP = mybir.AluOpType
    pay = st["pay"]
    # ---------------- sync-BN cross-core exchange ----------------
    cc_in = dramp.tile([C, 2], fp32)
    cc_gath = dramp.tile([N_CORES, C, 2], fp32)
    nc.gpsimd.dma_start(out=cc_in[:], in_=pay[:])
    if "noar" in ablate:
        for r in range(N_CORES):
            nc.gpsimd.dma_start(out=cc_gath[r], in_=cc_in[:])
    else:
        nc.gpsimd.collective_compute(
            "AllGather",
            OP.bypass,
            replica_groups=[list(range(N_CORES))],
            ins=[cc_in.opt()],
            outs=[cc_gath.opt()],
        )
    g_all = smallp.tile([C, N_CORES, 2], fp32)
    nc.gpsimd.dma_start(out=g_all[:], in_=cc_gath[:].rearrange("r p t -> p r t"))
    st["g_all"] = g_all


def _emit_back(nc, tc, pools, params, pre, st, pipelined=True, ablate=()):
    """Reduce + scale/bias chain + per-image quantize/conv/out."""
    fp32 = mybir.dt.float32
    xp, apadp, wp, tmpp, outp, smallp, psump, psmallp, dramp = pools
    x_d, gamma_d, beta_d, w_d, y_d = params
    AF = mybir.ActivationFunctionType
    OP = mybir.AluOpType
    ones, wq, alpha, gb, a_t = (pre["ones"], pre["wq"], pre["alpha"],
                                pre["gb"], pre["a_t"])
    xH, xgran, g_all = st["xH"], st["xgran"], st["g_all"]

    # local 8-way sum (same order on all cores), then exact /8
    g_sum = smallp.tile([C, 2], fp32)
    nc.vector.tensor_reduce(out=g_sum[:], in_=g_all[:].rearrange("p r t -> p t r"),
                            axis=mybir.AxisListType.X, op=OP.add)

    # PE fillers keep the p-state ramp hot through the collective window;
    # rhs depends on this iteration's first x tile so the scheduler cannot
    # hoist them into the previous iteration's conv burst
    ps_fill = pre["psm"][:, 384:512]
    if "nowarm" not in ablate:
        for _ in range(N_FILL):
            nc.tensor.matmul(ps_fill, lhsT=ones[:], rhs=xH[0][0][:, 0:128],
                             start=True, stop=True)

    # ---------------- global scale/bias ----------------
    # fused but bitwise-identical to the reference chain:
    # me = g_sum*0.125 (exact), vge = (E[x^2] - mean^2) + eps
    me = smallp.tile([C, 2], fp32)      # (global mean, global E[x^2])
    nc.vector.tensor_scalar_mul(me[:], g_sum[:], 0.125)
    meanv = me[:, 0:1]
    vge = smallp.tile([C, 1], fp32)     # var + eps
    gm2 = smallp.tile([C, 1], fp32)
    nc.vector.tensor_mul(gm2[:], meanv, meanv)
    nc.vector.tensor_scalar(vge[:], me[:, 1:2], gm2[:], BN_EPS,
                            OP.subtract, OP.add)
    rx = smallp.tile([C, 1], fp32)
    nc.scalar.activation(out=rx[:], in_=vge[:], func=AF.Sqrt)
    nc.vector.reciprocal(out=rx[:], in_=rx[:])
    tX = smallp.tile([C, 1], fp32)
    for _ in range(X_NEWTON):
        nc.vector.tensor_mul(tX[:], rx[:], rx[:])
        nc.vector.tensor_mul(tX[:], vge[:], tX[:])
        nc.vector.tensor_scalar(tX[:], tX[:], -0.5, 1.5, OP.mult, OP.add)
        nc.vector.tensor_mul(rx[:], rx[:], tX[:])

    # s = gamma * rsqrt / 0.538 ; b = (beta - mean*gamma*rsqrt) / 0.538
    s_q = smallp.tile([C, 1], fp32)
    b_q = smallp.tile([C, 1], fp32)
    ta = smallp.tile([C, 1], fp32)
    nc.vector.tensor_mul(ta[:], gb[:, 0:1], rx[:])          # A = gamma*inv
    nc.vector.tensor_scalar_mul(s_q[:], ta[:], 1.0 / HWGQ_STEP)
    tb = smallp.tile([C, 1], fp32)
    nc.vector.tensor_mul(tb[:], meanv, ta[:])               # mean*A
    nc.vector.tensor_scalar(b_q[:], gb[:, 1:2], tb[:], 1.0 / HWGQ_STEP,
                            OP.subtract, OP.mult)           # (beta-mean*A)/0.538

    # ---------------- per-image quantize + conv ----------------
    # all 28 affines first: x buffers release after ~10us instead of ~17us,
    # which is the binding recurrence for cross-iteration pipelining.
    # Early granules (g<3) on DVE (fast, feeds round chunk 0 quickly),
    # the rest on Pool.
    u_im = []
    for i in range(IMG):
        u_sb = tmpp.tile([C, S], fp32, tag="u", name=f"u_sb{i}")
        u_im.append(u_sb)
        for h in range(4):
            lo, hi = h * 896, min((h + 1) * 896, S)
            nc.vector.tensor_scalar(u_sb[:, lo:hi], xH[i][h][:],
                                    s_q[:], b_q[:], OP.mult, OP.add)
    for i in range(IMG):
        u_sb = u_im[i]
        # clip in place on Pool, then RNE round via MAGIC into the padded
        # fp8 tile (DVE; the last image's rounds go to Pool to shorten the
        # DVE tail, which is the binding engine in steady state)
        for (r0, r1) in ((0, 16), (16, 32), (32, 48), (48, 56)):
            lo, hi = r0 * HW, r1 * HW
            nc.gpsimd.tensor_scalar(u_sb[:, lo:hi], u_sb[:, lo:hi], 3.0, 0.0,
                                    OP.min, OP.max)
            reng = nc.vector if (i < 2 or (i == 2 and r0 < 32)) else nc.gpsimd
            reng.tensor_scalar(a_t[i][:, r0 + 1:r1 + 1, 2:58],
                               u_sb[:, lo:hi].rearrange(
                                   "p (h w) -> p h w", h=r1 - r0),
                               MAGIC, MAGIC, OP.add, OP.subtract)

        # bridge fillers: keep the PE p-state ramp alive across the
        # inter-image a_t dependency gap (dep on this image's clipped u);
        # only useful when iterations pipeline
        for _ in range(N_BRIDGE if pipelined else 0):
            nc.tensor.matmul(pre["psm"][:, 384:512], lhsT=ones[:],
                             rhs=u_sb[:, 0:128], start=True, stop=True)
        out_sb = outp.tile([C, S], mybir.dt.float16, tag="o", name=f"out_sb{i}")
        base = a_t[i][:]
        ps = [psump.tile([C, NFREE], fp32, tag=f"ps{c}", name=f"ps{i}_{c}")
              for c in range(NT)]
        # weight-stationary order (pass g outer, chunk cix inner): consecutive
        # matmuls of a pass share the same lhsT AP, so legalization skips the
        # Ldweights reload -- 5 LDWs per image instead of 35.  Passes:
        # 3 DoubleRow (kh=0&1 per kw), DoubleRow (2,0)+(2,1), single (2,2)
        if "noconv" not in ablate:
            for g in range(5):
                for cix in range(NT):
                    h0 = cix * R
                    if g < 3:
                        kw = g
                        rhs = bass.AP(
                            tensor=base.tensor,
                            offset=base.offset + h0 * PCW + (kw + 1),
                            ap=[base.ap[0], [PCW, 2], [PCW, R], [1, HW]],
                        )
                        nc.tensor.matmul(ps[cix][:], lhsT=wq[:, 2 * kw: 2 * kw + 2, :],
                                         rhs=rhs, start=(g == 0), stop=False,
                                         perf_mode=mybir.MatmulPerfMode.DoubleRow)
                    elif g == 3:
                        rhs = bass.AP(
                            tensor=base.tensor,
                            offset=base.offset + (h0 + 2) * PCW + 1,
                            ap=[base.ap[0], [1, 2], [PCW, R], [1, HW]],
                        )
                        nc.tensor.matmul(ps[cix][:], lhsT=wq[:, 6:8, :],
                                         rhs=rhs, start=False, stop=False,
                                         perf_mode=mybir.MatmulPerfMode.DoubleRow)
                    else:
                        rhs = a_t[i][:, h0 + 2: h0 + 2 + R, 3: 3 + HW]
                        nc.tensor.matmul(ps[cix][:], lhsT=wq[:, 8, :], rhs=rhs,
                                         start=False, stop=True)
                        # scale out of PSUM on ScalarE as soon as this bank's
                        # accumulation completes (gpsimd cannot read PSUM);
                        # fp16 output halves the out-DMA stream (<=2^-11 rel)
                        nc.scalar.activation(out=out_sb[:, h0 * HW: (h0 + R) * HW],
                                             in_=ps[cix][:], func=AF.Identity,
                                             scale=alpha[:])
                        # 896-col output DMAs (sync queue) to halve descriptors
                        if cix % 2 == 1 or cix == NT - 1:
                            olo = (cix // 2) * 2 * NFREE if cix % 2 == 1 else cix * NFREE
                            ohi = (cix + 1) * NFREE
                            nc.scalar.dma_start(out=y_d.ap()[i][:, olo:ohi],
                                                  in_=out_sb[:, olo:ohi])


def _build(n_iters=1, ablate=()):
    fp32 = mybir.dt.float32

    nc = bacc.Bacc("TRN2", target_bir_lowering=False, debug=False,
                   num_devices=N_CORES)

    x_d = nc.declare_dram_parameter("x", [IMG, C, S], fp32, isOutput=False)
    gamma_d = nc.declare_dram_parameter("gamma", [C], fp32, isOutput=False)
    beta_d = nc.declare_dram_parameter("beta", [C], fp32, isOutput=False)
    w_d = nc.declare_dram_parameter("weight", [C, 128 * 9], fp32, isOutput=False)
    y_d = nc.declare_dram_parameter("y", [IMG, C, S], mybir.dt.float16,
                                    isOutput=True)
    params = (x_d, gamma_d, beta_d, w_d, y_d)

    with tile.TileContext(nc) as tc:
        with (
            tc.tile_pool(name="xp", bufs=2) as xp,
            tc.tile_pool(name="apad", bufs=1) as apadp,
            tc.tile_pool(name="wp", bufs=1) as wp,
            tc.tile_pool(name="tmp", bufs=4) as tmpp,
            tc.tile_pool(name="outp", bufs=2) as outp,
            tc.tile_pool(name="small", bufs=1) as smallp,
            tc.tile_pool(name="psum", bufs=1, space="PSUM") as psump,
            tc.tile_pool(name="psmall", bufs=1, space="PSUM") as psmallp,
            tc.tile_pool(name="dram", bufs=4, space="DRAM") as dramp,
        ):
            pools = (xp, apadp, wp, tmpp, outp, smallp, psump, psmallp, dramp)
            pre = _emit_prelude(nc, tc, pools, params)
            st = _emit_front(nc, tc, pools, params, ablate)
            _emit_dispatch(nc, pools, st, ablate)
            for it in range(n_iters):
                nst = (_emit_front(nc, tc, pools, params, ablate)
                       if it + 1 < n_iters else None)
                _emit_back(nc, tc, pools, params, pre, st,
                           pipelined=n_iters > 1, ablate=ablate)
                if nst is not None:
                    _emit_dispatch(nc, pools, nst, ablate)
                st = nst

    nc.finalize()
    return nc


def _get_nc(n_iters=1):
    key = ("nc", n_iters)
    if key not in _CACHE:
        _CACHE[key] = _build(n_iters)
    return _CACHE[key]


def make_in_maps(x, gamma, beta, weight):
    x = np.ascontiguousarray(np.asarray(x, np.float32)).reshape(N_CORES, IMG, C, S)
    w = np.ascontiguousarray(np.asarray(weight, np.float32)).reshape(C, 128 * 9)
    gamma = np.ascontiguousarray(np.asarray(gamma, np.float32))
    beta = np.ascontiguousarray(np.asarray(beta, np.float32))
    return [
        {"x": x[c], "gamma": gamma, "beta": beta, "weight": w}
        for c in range(N_CORES)
    ]


def kernel(x, gamma, beta, weight):
    import os
    from concourse.bass_utils import run_bass_kernel_spmd

    nc = _get_nc()
    in_maps = make_in_maps(x, gamma, beta, weight)
    core_ids = list(range(N_CORES))
    try:
        res = run_bass_kernel_spmd(nc, in_maps, core_ids)
    except ModuleNotFoundError:
        # BASS_TRACE set but no NTFF profile hook in this container
        os.environ["BASS_NEVER_TRACE"] = "1"
        res = run_bass_kernel_spmd(nc, in_maps, core_ids)
    out = np.stack([res.results[c]["y"] for c in range(N_CORES)], axis=0)
    return out.reshape(32, C, HW, HW).astype(np.float32)

